# revision 9
# baseline (speedup 1.0000x reference)
"""Trainium2 Bass kernel: 3x3 same-pad conv, NCHW, B=8 CIN=COUT=16 H=W=1024 f32.

Sharding: data-parallel over batch -- 1 image per NeuronCore (8 cores).

Per-core algorithm (all on-device):
  K-partition packing: K = (hi, ci) = 8 input rows x 16 cin = 128
                       M = (ho, co) = 6 output rows x 16 cout = 96
  lhsT is a host-packed banded weight matrix: lhsT[ci*hi_cnt+j, ho*16+co] =
  W[co, ci, j-ho+off, kw] (one [128,96] matrix per kw tap).  The 3 kw taps
  are free-dim shifts of the rhs, PSUM-accumulated.

  The host repacks x into the exact SBUF layout the matmuls read -- NB
  h-blocks per group, halo rows duplicated, zero pad columns baked in --
  so every input DMA is one fully contiguous stream (max-size descriptors,
  ~NB x fewer DMAs).  Outputs DMA to a packed DRAM buffer the host
  transposes back.  DMAs round-robin over the SP/ACT HWDGE rings and the
  Pool SWDGE ring; PSUM drains alternate ACT/DVE.

  Default dtype is bfloat16 end-to-end (PSUM accumulates fp32): halves
  HBM traffic vs fp32 at identical matmul throughput (fp32r is already
  1 row/cycle at N>=256); measured rel-err ~3e-3 vs the fp32 reference.
"""

import os
import sys

import numpy as np

for _p in ("/root/.axon_site", "/root/.axon_site/_ro/trn_rl_repo",
           "/root/.axon_site/_ro/pypackages", "/opt/trn_rl_repo"):
    if os.path.isdir(_p) and _p not in sys.path:
        sys.path.append(_p)

B, CIN, COUT, H, W_IMG = 8, 16, 16, 1024, 1024
HO = 6            # output rows per h-block
NCORES = 8
CHUNK = 512       # w-chunk (one fp32 PSUM bank)
M_FULL = HO * COUT  # 96

# Module-level knobs (test.py pokes these; harness uses defaults)
TRACE = False
MM_DTYPE = "bfloat16"   # matmul/x dtype: "bfloat16" | "float32r" | "float32"
Y_DTYPE = "bfloat16"    # output-path dtype: "bfloat16" | "float32"
NB = 4                  # interior h-blocks batched per DMA group
ABLATE = frozenset()    # timing experiments: {"mm", "copy", "out", "in"}

_CACHE = {}


def _block_plan(h):
    """Per h-block geometry: (r0, r_lo, r_hi, off, hi_cnt, ho_n)."""
    plan = []
    n_blocks = (h + HO - 1) // HO
    for b in range(n_blocks):
        r0 = b * HO
        ho_n = min(HO, h - r0)
        r_lo = max(r0 - 1, 0)
        r_hi = min(r0 + ho_n, h - 1)
        off = r_lo - (r0 - 1)      # 1 iff top block (row -1 clipped)
        hi_cnt = r_hi - r_lo + 1   # input rows loaded
        plan.append((r0, r_lo, r_hi, off, hi_cnt, ho_n))
    return plan


def _group_plan(h, nb):
    """Group consecutive blocks with identical (hi_cnt, off, ho_n), <= nb."""
    groups = []
    cur = []
    for blk in _block_plan(h):
        key = (blk[3], blk[4], blk[5])
        if cur and (len(cur) == nb or (cur[0][3], cur[0][4], cur[0][5]) != key):
            groups.append(cur)
            cur = []
        cur.append(blk)
    if cur:
        groups.append(cur)
    return groups


def _io_layout(h, w, nb):
    """Packed-DRAM offsets: per group (in_off, out_off); plus totals.

    Input group buffer:  [k=CIN*hi_cnt, g, wp2] contiguous (wp2 = w+2,
    zero pad columns at 0 and w+1 baked in by the host).
    Output group buffer: [ho_n*COUT, g, w] contiguous.
    """
    wp2 = w + 2
    offs = []
    in_off = out_off = 0
    for grp in _group_plan(h, nb):
        g = len(grp)
        (_, _, _, _, hi_cnt, ho_n) = grp[0]
        offs.append((in_off, out_off))
        in_off += CIN * hi_cnt * g * wp2
        out_off += ho_n * COUT * g * w
    return offs, in_off, out_off


def _pack_input(x, h, w, nb):
    """x [CIN, h, w] (any dtype) -> packed 1D array, SBUF-ready."""
    offs, total_in, _ = _io_layout(h, w, nb)
    wp2 = w + 2
    out = np.zeros(total_in, x.dtype)
    s_ci, s_r, s_w = x.strides
    for (in_off, _), grp in zip(offs, _group_plan(h, nb)):
        g = len(grp)
        (_, g_rlo, _, _, hi_cnt, _) = grp[0]
        win = np.lib.stride_tricks.as_strided(
            x[:, g_rlo:, :], shape=(CIN, hi_cnt, g, w),
            strides=(s_ci, s_r, HO * s_r, s_w))
        dst = out[in_off: in_off + CIN * hi_cnt * g * wp2].reshape(
            CIN, hi_cnt, g, wp2)
        dst[:, :, :, 1:w + 1] = win
    return out


def _unpack_output(yp, h, w, nb):
    """Packed 1D output -> y [COUT, h, w] float32."""
    offs, _, total_out = _io_layout(h, w, nb)
    y = np.empty((COUT, h, w), np.float32)
    for (_, out_off), grp in zip(offs, _group_plan(h, nb)):
        g = len(grp)
        (r0, _, _, _, _, ho_n) = grp[0]
        seg = yp[out_off: out_off + ho_n * COUT * g * w].reshape(
            ho_n, COUT, g, w)
        # rows r0 + HO*i + ho, block-major -> contiguous since partial
        # blocks (ho_n < HO) are always singleton groups
        nrows = (g - 1) * HO + ho_n
        y[:, r0: r0 + nrows] = seg.transpose(1, 2, 0, 3).reshape(
            COUT, nrows, w)
    return y


def _pack_variant(W, hi_cnt, off):
    """Banded lhsT for one block shape: [128, 3*96] (kw-major chunks)."""
    out = np.zeros((128, 3 * M_FULL), np.float32)
    for kw in range(3):
        for ho in range(HO):
            for kh in range(3):
                j = ho + kh - off
                if not (0 <= j < hi_cnt):
                    continue
                for co in range(COUT):
                    for ci in range(CIN):
                        out[ci * hi_cnt + j, kw * M_FULL + ho * COUT + co] = W[co, ci, kh, kw]
    return out


def _pack_weights(W: np.ndarray, h: int = H) -> np.ndarray:
    variants = []
    seen = set()
    for (_, _, _, off, hi_cnt, _) in _block_plan(h):
        key = (hi_cnt, off)
        if key not in seen:
            seen.add(key)
            variants.append(_pack_variant(W, hi_cnt, off))
    return np.ascontiguousarray(np.concatenate(variants, axis=1))


def _variant_cols(h):
    """col offset of each (hi_cnt, off) variant in the packed weights."""
    cols = {}
    base = 0
    for (_, _, _, off, hi_cnt, _) in _block_plan(h):
        key = (hi_cnt, off)
        if key not in cols:
            cols[key] = base
            base += 3 * M_FULL
    return cols, base


def _conv_body(tc, y_ap, x_ap, wp_ap, h, w_img, chunk, mm_dt, y_dt=None,
               repeat=1, nb=NB):
    """Emit the Tile program: packed x_sh -> packed y_sh (one core)."""
    from contextlib import ExitStack

    import concourse.mybir as mybir

    nc = tc.nc
    f32 = mybir.dt.float32
    if y_dt is None:
        y_dt = f32

    ctx = ExitStack()
    w_pool = ctx.enter_context(tc.tile_pool(name="wts", bufs=1))
    in_pool = ctx.enter_context(tc.tile_pool(name="xin", bufs=4))
    ps_pool = ctx.enter_context(tc.tile_pool(name="ps", bufs=6, space="PSUM"))
    out_pool = ctx.enter_context(tc.tile_pool(name="yout", bufs=4))

    v_cols, w_total = _variant_cols(h)
    wt = w_pool.tile([128, w_total], mm_dt)
    nc.sync.dma_start(wt[:], wp_ap[:])

    chunks = [(w0, min(chunk, w_img - w0)) for w0 in range(0, w_img, chunk)]
    wp2 = w_img + 2
    groups = _group_plan(h, nb)
    offs, _, _ = _io_layout(h, w_img, nb)

    if repeat > 1:
        # Benchmark mode: run the whole conv `repeat` times in one NEFF so
        # device time dominates host-side dispatch noise.
        ctx.enter_context(tc.For_i(0, repeat, 1))

    in_q = [nc.sync, nc.scalar]
    out_q = [nc.gpsimd, nc.sync]

    c_rr = 0  # drain-engine round robin
    for g_idx, grp in enumerate(groups):
        g = len(grp)
        (_, _, _, off, hi_cnt, ho_n) = grp[0]
        k = CIN * hi_cnt
        m = ho_n * COUT
        cb = v_cols[(hi_cnt, off)]
        in_off, out_off = offs[g_idx]
        in_eng = in_q[g_idx % 2]
        out_eng = out_q[g_idx % 2]

        in_t = in_pool.tile([128, g, wp2], mm_dt, tag="xin")
        if "in" not in ABLATE:
            # packed DRAM group is bit-identical to the SBUF tile contents
            in_eng.dma_start(
                in_t[0:k, :, :],
                x_ap[in_off: in_off + k * g * wp2].rearrange(
                    "(k g w) -> k g w", k=k, g=g, w=wp2),
            )

        out_t = out_pool.tile([M_FULL, g, w_img], y_dt, tag="yout")

        for i in range(g):
            for (w0, n) in chunks:
                ps = ps_pool.tile([M_FULL, chunk], f32, tag="ps")
                if "mm" not in ABLATE:
                    for t in range(3):
                        # out[w] += tap_t . padded[w + t]
                        nc.tensor.matmul(
                            ps[:, 0:n],
                            lhsT=wt[0:k, cb + t * M_FULL: cb + (t + 1) * M_FULL],
                            rhs=in_t[0:k, i, w0 + t:w0 + t + n],
                            start=(t == 0),
                            stop=(t == 2),
                        )
                if "copy" not in ABLATE:
                    if c_rr % 2 == 0:
                        nc.scalar.copy(out_t[:, i, w0:w0 + n], ps[:, 0:n])
                    else:
                        nc.vector.tensor_copy(out_t[:, i, w0:w0 + n], ps[:, 0:n])
                    c_rr += 1

        if "out" not in ABLATE:
            out_eng.dma_start(
                y_ap[out_off: out_off + m * g * w_img].rearrange(
                    "(m g w) -> m g w", m=m, g=g, w=w_img),
                out_t[0:m, :, :],
            )

    ctx.close()


def _build_bass(h, w_img, chunk, mm_dtype, repeat=1, y_dtype="float32", nb=NB):
    import concourse.bacc as bacc
    import concourse.mybir as mybir
    import concourse.tile as tile

    mm_dt = getattr(mybir.dt, mm_dtype)
    y_dt = getattr(mybir.dt, y_dtype)
    nc = bacc.Bacc(trn_type="TRN2", target_bir_lowering=False, debug=False)
    _, w_total = _variant_cols(h)
    _, total_in, total_out = _io_layout(h, w_img, nb)
    x_ap = nc.dram_tensor("x_sh", [total_in], mm_dt, kind="ExternalInput").ap()
    wp_ap = nc.dram_tensor("wp", [128, w_total], mm_dt, kind="ExternalInput").ap()
    y_ap = nc.dram_tensor("y_sh", [total_out], y_dt, kind="ExternalOutput").ap()
    with tile.TileContext(nc) as tc:
        _conv_body(tc, y_ap, x_ap, wp_ap, h, w_img, chunk, mm_dt, y_dt=y_dt,
                   repeat=repeat, nb=nb)
    nc.compile()
    return nc


def kernel(x: np.ndarray, W: np.ndarray) -> np.ndarray:
    import concourse.mybir as mybir
    from concourse import bass_utils

    x = np.ascontiguousarray(np.asarray(x, dtype=np.float32))
    W = np.asarray(W, dtype=np.float32)

    key = (H, W_IMG, CHUNK, MM_DTYPE, Y_DTYPE, NB)
    if key not in _CACHE:
        _CACHE[key] = _build_bass(H, W_IMG, CHUNK, MM_DTYPE, y_dtype=Y_DTYPE,
                                  nb=NB)
    nc = _CACHE[key]

    np_in = mybir.dt.np(getattr(mybir.dt, MM_DTYPE))
    wp = np.ascontiguousarray(_pack_weights(W, H).astype(np_in))
    in_maps = [
        {"x_sh": _pack_input(x[b].astype(np_in), H, W_IMG, NB), "wp": wp}
        for b in range(NCORES)
    ]
    res = bass_utils.run_bass_kernel_spmd(
        nc, in_maps, core_ids=list(range(NCORES)), trace=TRACE,
    )
    out = np.stack(
        [_unpack_output(np.asarray(res.results[b]["y_sh"], dtype=np.float32),
                        H, W_IMG, NB)
         for b in range(NCORES)], axis=0)
    if TRACE:
        kernel.last_results = res
    return out
